# revision 18
# baseline (speedup 1.0000x reference)
"""MeshUnpool Trainium2 kernel (v2 — windowed scatter + dma_gather).

out[s] = x_coarse[argmin_j |keep_idx[j] - s|] for every fine slot s
(first-min tie-break; kept slots resolve to their own j since dist 0).

Each of the 8 cores fills its 2048-slot slice using an 18-partition x 128
window table (own 2048 slots + 128-slot halo each side; max kept-gap in the
data is 14, so the halo is safe and asserted on host).

Device pipeline per core:
  1. Scatter build: host pre-buckets the window's keep entries by
     f = slot&127 (row) with bucket capacity B=16.  A [128,16,18] one-hot
     compare of the bucketed hi' values against the 18 window partitions,
     times payloads (j>>6)+1 and j&63, then a log-tree sum over the bucket
     axis yields T^T[f, p'] — the slot table transposed (at most one entry
     per (f,p'), so bf16 sums are exact).
  2. One PE transpose -> T[p', f]; build f32 keys
     key1 = kept*(128*pos + j_hi), key2 = kept*(64*pos + j_lo); run
     prefix-max / suffix-min scans along slots with a transposed
     cross-partition carry fixup; decode nearest-left/right and tie-break
     to src_j per slot.
  3. The own-slot block [16,128] of src is shifted to partitions 0:16
     (tiny SBUF->SBUF DMA) and fed as int16 indices to 4 dma_gather
     custom instructions (512 rows each, ~0.34ns/descriptor SWDGE) that
     pull 2KB rows straight out of DRAM x_coarse; 16 strided dma_starts
     write the rows to their slots in y.

Inputs are replicated; only the small index-prep tensors differ per core.
"""

import os
import sys

import numpy as np

E_FINE = 16384
E_COARSE = 8192
C = 512
N_CORES = 8
P = 128
SLICE = E_FINE // N_CORES  # 2048
B = 16       # bucket capacity per (core, f) — data max is 15
U = 18       # window partitions = 2048/128 + 2 halo
R_SENT = 8388608.0  # 2^23 sentinel for suffix-min scans (> any key)

_NC_CACHE = {}


def _ensure_paths():
    for p in ("/opt/trn_rl_repo", "/root/.axon_site/_ro/trn_rl_repo"):
        if os.path.isdir(p) and p not in sys.path:
            sys.path.append(p)


def build_program(nc, bass, mybir, tile):
    f32 = mybir.dt.float32
    i32 = mybir.dt.int32
    i16 = mybir.dt.int16
    bf16 = mybir.dt.bfloat16
    Alu = mybir.AluOpType

    xc = nc.dram_tensor("xc", [E_COARSE, C], f32, kind="ExternalInput")
    # pk[:, 0:16]=hi' (255 = empty), [:,16:32]=(j>>6)+1, [:,32:48]=j&63,
    # [:,48:66]=iota over the 18 window partitions (same row everywhere)
    pk = nc.dram_tensor("pk", [P, 3 * B + U], bf16, kind="ExternalInput")
    posw = nc.dram_tensor("posw", [U, P], i32, kind="ExternalInput")
    # y[rw, cg, rv, :] = output row for slot 128*rw + 8*cg + rv
    y = nc.dram_tensor("y", [16, 16, 8, C], f32, kind="ExternalOutput")

    with tile.TileContext(nc) as tc:
        with (
            tc.tile_pool(name="sb", bufs=1) as sb,
            tc.tile_pool(name="ps", bufs=1, space="PSUM") as ps,
        ):
            pk_t = sb.tile([P, 3 * B + U], bf16)
            nc.sync.dma_start(pk_t[:], pk[:])
            pos_t = sb.tile([U, P], i32)
            nc.sync.dma_start(pos_t[:], posw[:])
            idf_t = sb.tile([P, P], f32)
            nc.gpsimd.memset(idf_t[:], 1.0)
            nc.gpsimd.affine_select(
                out=idf_t[:],
                in_=idf_t[:],
                compare_op=Alu.is_equal,
                fill=0.0,
                base=0,
                pattern=[[-1, P]],
                channel_multiplier=1,
            )

            hi_b = pk_t[:, 0:B]
            v1 = pk_t[:, B : 2 * B]
            v2 = pk_t[:, 2 * B : 3 * B]
            iotau = pk_t[:, 3 * B : 3 * B + U]

            # one-hot of hi' against the window partitions, with payloads
            onehot = sb.tile([P, B, U], bf16)
            nc.vector.tensor_tensor(
                onehot[:],
                hi_b.unsqueeze(2).to_broadcast([P, B, U]),
                iotau.unsqueeze(1).to_broadcast([P, B, U]),
                Alu.is_equal,
            )
            w = sb.tile([P, B, 2 * U], bf16)
            nc.vector.tensor_tensor(
                w[:, :, 0:U],
                onehot[:],
                v1.unsqueeze(2).to_broadcast([P, B, U]),
                Alu.mult,
            )
            nc.vector.tensor_tensor(
                w[:, :, U : 2 * U],
                onehot[:],
                v2.unsqueeze(2).to_broadcast([P, B, U]),
                Alu.mult,
            )
            # log-tree sum over the bucket axis (exact: <=1 nonzero per lane)
            h = B // 2
            while h >= 2:
                nc.vector.tensor_tensor(
                    w[:, 0:h, :], w[:, 0:h, :], w[:, h : 2 * h, :], Alu.add
                )
                h //= 2
            wf = sb.tile([P, 2 * U], f32)
            nc.vector.tensor_tensor(
                wf[:].unsqueeze(1), w[:, 0:1, :], w[:, 1:2, :], Alu.add
            )

            # T^T[f, 0:U]=jhi1 sums, [f, U:2U]=jlo sums -> transpose each to
            # its own [U, f] tile (sliced partition bases must be 32-aligned)
            th_ps = ps.tile([U, P], f32)
            nc.tensor.transpose(th_ps[:], wf[:, 0:U], idf_t[:])
            tl_ps = ps.tile([U, P], f32)
            nc.tensor.transpose(tl_ps[:], wf[:, U : 2 * U], idf_t[:])
            ttab = sb.tile([U, P], f32)
            nc.vector.tensor_copy(ttab[:], th_ps[:])
            tlow = sb.tile([U, P], f32)
            nc.scalar.copy(tlow[:], tl_ps[:])
            th_s = ttab[:]
            tl_s = tlow[:]

            # keys
            mk = sb.tile([U, P], f32)
            nc.vector.tensor_scalar(mk[:], th_s, 0.0, None, Alu.is_gt)
            th1 = sb.tile([U, P], f32)
            nc.vector.tensor_scalar(th1[:], th_s, 1.0, None, Alu.subtract)
            posf = sb.tile([U, P], f32)
            nc.scalar.copy(posf[:], pos_t[:])
            k1r = sb.tile([U, P], f32)
            nc.vector.scalar_tensor_tensor(
                k1r[:], posf[:], 128.0, th1[:], Alu.mult, Alu.add
            )
            key1 = sb.tile([U, P], f32)
            nc.vector.tensor_tensor(key1[:], k1r[:], mk[:], Alu.mult)
            k2r = sb.tile([U, P], f32)
            nc.vector.scalar_tensor_tensor(
                k2r[:], posf[:], 64.0, tl_s, Alu.mult, Alu.add
            )
            key2 = sb.tile([U, P], f32)
            nc.gpsimd.tensor_tensor(key2[:], k2r[:], mk[:], Alu.mult)
            msk = sb.tile([U, P], f32)
            nc.vector.tensor_scalar(msk[:], key1[:], 0.0, None, Alu.is_equal)
            r1 = sb.tile([U, P], f32)
            nc.vector.scalar_tensor_tensor(
                r1[:], msk[:], R_SENT, key1[:], Alu.mult, Alu.add
            )
            r2 = sb.tile([U, P], f32)
            nc.vector.scalar_tensor_tensor(
                r2[:], msk[:], R_SENT, key2[:], Alu.mult, Alu.add
            )

            # per-partition scans along the slot (free) axis
            l1s = sb.tile([U, P], f32)
            nc.vector.tensor_tensor_scan(
                l1s[:], key1[:], key1[:], 0.0, Alu.max, Alu.max
            )
            l2s = sb.tile([U, P], f32)
            nc.vector.tensor_tensor_scan(
                l2s[:], key2[:], key2[:], 0.0, Alu.max, Alu.max
            )
            r1s = sb.tile([U, P], f32)
            nc.vector.tensor_tensor_scan(
                r1s[:, P - 1 :: -1],
                r1[:, P - 1 :: -1],
                r1[:, P - 1 :: -1],
                R_SENT,
                Alu.min,
                Alu.min,
            )
            r2s = sb.tile([U, P], f32)
            nc.vector.tensor_tensor_scan(
                r2s[:, P - 1 :: -1],
                r2[:, P - 1 :: -1],
                r2[:, P - 1 :: -1],
                R_SENT,
                Alu.min,
                Alu.min,
            )

            # cross-partition carry via transposed exclusive scans
            totl = sb.tile([U, 2], f32)
            nc.vector.tensor_copy(totl[:, 0:1], l1s[:, P - 1 : P])
            nc.vector.tensor_copy(totl[:, 1:2], l2s[:, P - 1 : P])
            totr = sb.tile([U, 2], f32)
            nc.gpsimd.tensor_copy(totr[:, 0:1], r1s[:, 0:1])
            nc.gpsimd.tensor_copy(totr[:, 1:2], r2s[:, 0:1])
            tl_ps2 = ps.tile([2, U], f32)
            nc.tensor.transpose(tl_ps2[:], totl[:], idf_t[0:U, 0:U])
            tr_ps = ps.tile([2, U], f32)
            nc.tensor.transpose(tr_ps[:], totr[:], idf_t[0:U, 0:U])
            tlt = sb.tile([2, U], f32)
            nc.vector.tensor_copy(tlt[:], tl_ps2[:])
            trt = sb.tile([2, U], f32)
            nc.scalar.copy(trt[:], tr_ps[:])
            exl = sb.tile([2, U], f32)
            nc.vector.memset(exl[:, 0:1], 0.0)
            nc.vector.tensor_tensor_scan(
                exl[:, 1:U],
                tlt[:, 0 : U - 1],
                tlt[:, 0 : U - 1],
                0.0,
                Alu.max,
                Alu.max,
            )
            exr = sb.tile([2, U], f32)
            nc.vector.memset(exr[:, U - 1 : U], R_SENT)
            nc.vector.tensor_tensor_scan(
                exr[:, U - 2 :: -1],
                trt[:, U - 1 : 0 : -1],
                trt[:, U - 1 : 0 : -1],
                R_SENT,
                Alu.min,
                Alu.min,
            )
            cl_ps = ps.tile([U, 2], f32)
            nc.tensor.transpose(cl_ps[:], exl[:], idf_t[0:2, 0:2])
            cr_ps = ps.tile([U, 2], f32)
            nc.tensor.transpose(cr_ps[:], exr[:], idf_t[0:2, 0:2])
            cl = sb.tile([U, 2], f32)
            nc.vector.tensor_copy(cl[:], cl_ps[:])
            cr = sb.tile([U, 2], f32)
            nc.scalar.copy(cr[:], cr_ps[:])
            nc.vector.tensor_scalar_max(l1s[:], l1s[:], cl[:, 0:1])
            nc.vector.tensor_scalar_max(l2s[:], l2s[:], cl[:, 1:2])
            nc.gpsimd.tensor_scalar_min(r1s[:], r1s[:], cr[:, 0:1])
            nc.gpsimd.tensor_scalar_min(r2s[:], r2s[:], cr[:, 1:2])

            # decode: slot = key1>>7, j = ((key1&127)<<6) | (key2&63)
            l1i = sb.tile([U, P], i32)
            nc.vector.tensor_copy(l1i[:], l1s[:])
            l2i = sb.tile([U, P], i32)
            nc.vector.tensor_copy(l2i[:], l2s[:])
            r1i = sb.tile([U, P], i32)
            nc.gpsimd.tensor_copy(r1i[:], r1s[:])
            r2i = sb.tile([U, P], i32)
            nc.gpsimd.tensor_copy(r2i[:], r2s[:])

            slot_l = sb.tile([U, P], i32)
            nc.vector.tensor_scalar(slot_l[:], l1i[:], 7, None, Alu.arith_shift_right)
            slot_r = sb.tile([U, P], i32)
            nc.vector.tensor_scalar(slot_r[:], r1i[:], 7, None, Alu.arith_shift_right)
            jhl = sb.tile([U, P], i32)
            nc.vector.tensor_scalar(
                jhl[:], l1i[:], 127, 6, Alu.bitwise_and, Alu.arith_shift_left
            )
            jll = sb.tile([U, P], i32)
            nc.vector.tensor_scalar(jll[:], l2i[:], 63, None, Alu.bitwise_and)
            jl = sb.tile([U, P], i32)
            nc.vector.tensor_tensor(jl[:], jhl[:], jll[:], Alu.bitwise_or)
            jhr = sb.tile([U, P], i32)
            nc.vector.tensor_scalar(
                jhr[:], r1i[:], 127, 6, Alu.bitwise_and, Alu.arith_shift_left
            )
            jlr = sb.tile([U, P], i32)
            nc.vector.tensor_scalar(jlr[:], r2i[:], 63, None, Alu.bitwise_and)
            jr = sb.tile([U, P], i32)
            nc.vector.tensor_tensor(jr[:], jhr[:], jlr[:], Alu.bitwise_or)

            dl = sb.tile([U, P], i32)
            nc.vector.tensor_tensor(dl[:], pos_t[:], slot_l[:], Alu.subtract)
            drr = sb.tile([U, P], i32)
            nc.gpsimd.tensor_tensor(drr[:], slot_r[:], pos_t[:], Alu.subtract)
            m_l = sb.tile([U, P], i32)
            nc.vector.tensor_tensor(m_l[:], dl[:], drr[:], Alu.is_lt)
            m_r = sb.tile([U, P], i32)
            nc.vector.tensor_tensor(m_r[:], drr[:], dl[:], Alu.is_lt)
            src = sb.tile([U, P], i32)
            nc.vector.tensor_tensor(src[:], jl[:], jr[:], Alu.min)
            nc.vector.copy_predicated(src[:], m_r[:], jr[:])
            nc.vector.copy_predicated(src[:], m_l[:], jl[:])

            # own slots live on partitions 1..16; shift to 0..15 via a PE
            # transpose pair (free-dim slice of the transposed table)
            srcf = sb.tile([U, P], f32)
            nc.vector.tensor_copy(srcf[:], src[:])
            g_ps = ps.tile([P, U], f32)
            nc.tensor.transpose(g_ps[:], srcf[:], idf_t[0:U, 0:U])
            g_sb = sb.tile([P, U], f32)
            nc.vector.tensor_copy(g_sb[:], g_ps[:])
            # dma_gather's 8 DSP cores each read their own 16-partition group:
            # replicate the 16 own-slot columns 8x, then one transpose lands
            # idxs[16c+p, s] = src[p+1, s] for every DSP core c
            g_rep = sb.tile([P, 8, 16], f32)
            nc.vector.tensor_copy(
                g_rep[:], g_sb[:, 1:17].unsqueeze(1).to_broadcast([P, 8, 16])
            )
            ix_ps = ps.tile([P, P], f32)
            nc.tensor.transpose(
                ix_ps[:], g_rep[:].rearrange("a b c -> a (b c)"), idf_t[:]
            )
            idxs = sb.tile([P, P], i16)
            nc.vector.tensor_copy(idxs[:], ix_ps[:])

            # gather + write out, 4 pipelined quarters
            NQ = 4
            SQ = P // NQ  # 32 idx columns -> 512 rows per quarter
            for q in range(NQ):
                gt = sb.tile([P, 4, C], f32, tag=f"g{q}")
                nc.gpsimd.dma_gather(
                    out_ap=gt[:],
                    in_ap=xc[:],
                    idxs_ap=idxs[:, q * SQ : (q + 1) * SQ],
                    num_idxs=4 * P,
                    num_idxs_reg=4 * P,
                    elem_size=C,
                    queue_num=q % 2,
                )
                for cc in range(4):
                    cg = 4 * q + cc
                    weng = nc.sync if cc % 2 == 0 else nc.scalar
                    weng.dma_start(
                        y[:, cg : cg + 1, :, :].squeeze(1).transpose([1, 0, 2]),
                        gt[:, cc, :],
                    )

    return {"y": y}


def host_inputs(x_coarse, keep_idx):
    import ml_dtypes

    bf = ml_dtypes.bfloat16
    x_coarse = np.ascontiguousarray(np.asarray(x_coarse), dtype=np.float32)
    keep = np.ascontiguousarray(np.asarray(keep_idx)).astype(np.int64).reshape(-1)

    iotau = np.tile(np.arange(U, dtype=np.float64), (P, 1))

    in_maps = []
    for m in range(N_CORES):
        base = SLICE * m - P
        hi = (keep >> 7) - (16 * m - 1)
        jj = np.nonzero((hi >= 0) & (hi < U))[0]
        f = (keep[jj] & 127).astype(np.int64)
        order = np.argsort(f, kind="stable")
        fo = f[order]
        jo = jj[order]
        cnt = np.bincount(fo, minlength=P)
        assert cnt.max() <= B, f"bucket overflow: {cnt.max()} > {B}"
        start = np.zeros(P, dtype=np.int64)
        start[1:] = np.cumsum(cnt)[:-1]
        slot_in_bucket = np.arange(len(jo)) - start[fo]
        hi_b = np.full((P, B), 255.0)
        v1 = np.zeros((P, B))
        v2 = np.zeros((P, B))
        hi_b[fo, slot_in_bucket] = hi[jo]
        v1[fo, slot_in_bucket] = (jo >> 6) + 1
        v2[fo, slot_in_bucket] = jo & 63
        pk = np.concatenate([hi_b, v1, v2, iotau], axis=1).astype(bf)
        pos = (
            16384
            + base
            + 128 * np.arange(U, dtype=np.int64)[:, None]
            + np.arange(P, dtype=np.int64)[None, :]
        ).astype(np.int32)
        in_maps.append(
            {
                "xc": x_coarse,
                "pk": np.ascontiguousarray(pk),
                "posw": np.ascontiguousarray(pos),
            }
        )
    return in_maps


def _get_nc():
    if "nc" in _NC_CACHE:
        return _NC_CACHE["nc"]
    _ensure_paths()
    from concourse import bass, mybir
    import concourse.bacc as bacc
    import concourse.tile as tile

    nc = bacc.Bacc(
        "TRN2",
        target_bir_lowering=False,
        debug=False,
        dynamic_dma_scratch_size=16384,
        num_swdge_queues=2,
    )
    build_program(nc, bass, mybir, tile)
    nc.compile()
    _NC_CACHE["nc"] = nc
    return nc


def run_on_hw(in_maps, trace=False, **kwargs):
    _ensure_paths()
    from concourse.bass_utils import run_bass_kernel_spmd

    nc = _get_nc()
    return run_bass_kernel_spmd(
        nc, in_maps, core_ids=list(range(N_CORES)), trace=trace, **kwargs
    )


def kernel(x_coarse, keep_idx, E_fine=None, **_unused):
    in_maps = host_inputs(x_coarse, keep_idx)
    res = run_on_hw(in_maps)
    out = np.concatenate(
        [res.results[m]["y"].reshape(SLICE, C) for m in range(N_CORES)], axis=0
    )
    return np.ascontiguousarray(out.astype(np.float32, copy=False))


# revision 19
# speedup vs baseline: 1.1654x; 1.1654x over previous
"""MeshUnpool Trainium2 kernel (v2 — windowed scatter + dma_gather).

out[s] = x_coarse[argmin_j |keep_idx[j] - s|] for every fine slot s
(first-min tie-break; kept slots resolve to their own j since dist 0).

Each of the 8 cores fills its 2048-slot slice using an 18-partition x 128
window table (own 2048 slots + 128-slot halo each side; max kept-gap in the
data is 14, so the halo is safe and asserted on host).

Device pipeline per core:
  1. Scatter build: host pre-buckets the window's keep entries by
     f = slot&127 (row) with bucket capacity B=16.  A [128,16,18] one-hot
     compare of the bucketed hi' values against the 18 window partitions,
     times payloads (j>>6)+1 and j&63, then a log-tree sum over the bucket
     axis yields T^T[f, p'] — the slot table transposed (at most one entry
     per (f,p'), so bf16 sums are exact).
  2. One PE transpose -> T[p', f]; build f32 keys
     key1 = kept*(128*pos + j_hi), key2 = kept*(64*pos + j_lo); run
     prefix-max / suffix-min scans along slots with a transposed
     cross-partition carry fixup; decode nearest-left/right and tie-break
     to src_j per slot.
  3. The own-slot block [16,128] of src is shifted to partitions 0:16
     (tiny SBUF->SBUF DMA) and fed as int16 indices to 4 dma_gather
     custom instructions (512 rows each, ~0.34ns/descriptor SWDGE) that
     pull 2KB rows straight out of DRAM x_coarse; 16 strided dma_starts
     write the rows to their slots in y.

Inputs are replicated; only the small index-prep tensors differ per core.
"""

import os
import sys

import numpy as np

E_FINE = 16384
E_COARSE = 8192
C = 512
N_CORES = 8
P = 128
SLICE = E_FINE // N_CORES  # 2048
B = 16       # bucket capacity per (core, f) — data max is 15
U = 18       # window partitions = 2048/128 + 2 halo
R_SENT = 8388608.0  # 2^23 sentinel for suffix-min scans (> any key)

_NC_CACHE = {}


def _ensure_paths():
    for p in ("/opt/trn_rl_repo", "/root/.axon_site/_ro/trn_rl_repo"):
        if os.path.isdir(p) and p not in sys.path:
            sys.path.append(p)


def build_program(nc, bass, mybir, tile):
    f32 = mybir.dt.float32
    i32 = mybir.dt.int32
    i16 = mybir.dt.int16
    bf16 = mybir.dt.bfloat16
    Alu = mybir.AluOpType

    xc = nc.dram_tensor("xc", [E_COARSE, C], f32, kind="ExternalInput")
    # pk[:, 0:16]=hi' (255 = empty), [:,16:32]=(j>>6)+1, [:,32:48]=j&63,
    # [:,48:66]=iota over the 18 window partitions (same row everywhere)
    pk = nc.dram_tensor("pk", [P, 3 * B + U], bf16, kind="ExternalInput")
    posw = nc.dram_tensor("posw", [U, P], i32, kind="ExternalInput")
    # y[rw, cg, rv, :] = output row for slot 128*rw + 8*cg + rv
    y = nc.dram_tensor("y", [16, 16, 8, C], f32, kind="ExternalOutput")

    with tile.TileContext(nc) as tc:
        with (
            tc.tile_pool(name="sb", bufs=1) as sb,
            tc.tile_pool(name="ps", bufs=1, space="PSUM") as ps,
        ):
            pk_t = sb.tile([P, 3 * B + U], bf16)
            nc.sync.dma_start(pk_t[:], pk[:])
            pos_t = sb.tile([U, P], i32)
            nc.sync.dma_start(pos_t[:], posw[:])
            idf_t = sb.tile([P, P], f32)
            nc.gpsimd.memset(idf_t[:], 1.0)
            nc.gpsimd.affine_select(
                out=idf_t[:],
                in_=idf_t[:],
                compare_op=Alu.is_equal,
                fill=0.0,
                base=0,
                pattern=[[-1, P]],
                channel_multiplier=1,
            )

            hi_b = pk_t[:, 0:B]
            v1 = pk_t[:, B : 2 * B]
            v2 = pk_t[:, 2 * B : 3 * B]
            iotau = pk_t[:, 3 * B : 3 * B + U]

            # one-hot of hi' against the window partitions, with payloads
            onehot = sb.tile([P, B, U], bf16)
            nc.vector.tensor_tensor(
                onehot[:],
                hi_b.unsqueeze(2).to_broadcast([P, B, U]),
                iotau.unsqueeze(1).to_broadcast([P, B, U]),
                Alu.is_equal,
            )
            w = sb.tile([P, B, 2 * U], bf16)
            nc.vector.tensor_tensor(
                w[:, :, 0:U],
                onehot[:],
                v1.unsqueeze(2).to_broadcast([P, B, U]),
                Alu.mult,
            )
            nc.vector.tensor_tensor(
                w[:, :, U : 2 * U],
                onehot[:],
                v2.unsqueeze(2).to_broadcast([P, B, U]),
                Alu.mult,
            )
            # log-tree sum over the bucket axis (exact: <=1 nonzero per lane)
            h = B // 2
            while h >= 2:
                nc.vector.tensor_tensor(
                    w[:, 0:h, :], w[:, 0:h, :], w[:, h : 2 * h, :], Alu.add
                )
                h //= 2
            wf = sb.tile([P, 2 * U], f32)
            nc.vector.tensor_tensor(
                wf[:].unsqueeze(1), w[:, 0:1, :], w[:, 1:2, :], Alu.add
            )

            # T^T[f, 0:U]=jhi1 sums, [f, U:2U]=jlo sums -> transpose each to
            # its own [U, f] tile (sliced partition bases must be 32-aligned)
            th_ps = ps.tile([U, P], f32)
            nc.tensor.transpose(th_ps[:], wf[:, 0:U], idf_t[:])
            tl_ps = ps.tile([U, P], f32)
            nc.tensor.transpose(tl_ps[:], wf[:, U : 2 * U], idf_t[:])
            ttab = sb.tile([U, P], f32)
            nc.vector.tensor_copy(ttab[:], th_ps[:])
            tlow = sb.tile([U, P], f32)
            nc.vector.tensor_copy(tlow[:], tl_ps[:])
            th_s = ttab[:]
            tl_s = tlow[:]

            # keys
            mk = sb.tile([U, P], f32)
            nc.vector.tensor_scalar(mk[:], th_s, 0.0, None, Alu.is_gt)
            th1 = sb.tile([U, P], f32)
            nc.vector.tensor_scalar(th1[:], th_s, 1.0, None, Alu.subtract)
            posf = sb.tile([U, P], f32)
            nc.vector.tensor_copy(posf[:], pos_t[:])
            k1r = sb.tile([U, P], f32)
            nc.vector.scalar_tensor_tensor(
                k1r[:], posf[:], 128.0, th1[:], Alu.mult, Alu.add
            )
            key1 = sb.tile([U, P], f32)
            nc.vector.tensor_tensor(key1[:], k1r[:], mk[:], Alu.mult)
            k2r = sb.tile([U, P], f32)
            nc.vector.scalar_tensor_tensor(
                k2r[:], posf[:], 64.0, tl_s, Alu.mult, Alu.add
            )
            key2 = sb.tile([U, P], f32)
            nc.vector.tensor_tensor(key2[:], k2r[:], mk[:], Alu.mult)
            msk = sb.tile([U, P], f32)
            nc.vector.tensor_scalar(msk[:], key1[:], 0.0, None, Alu.is_equal)
            r1 = sb.tile([U, P], f32)
            nc.vector.scalar_tensor_tensor(
                r1[:], msk[:], R_SENT, key1[:], Alu.mult, Alu.add
            )
            r2 = sb.tile([U, P], f32)
            nc.vector.scalar_tensor_tensor(
                r2[:], msk[:], R_SENT, key2[:], Alu.mult, Alu.add
            )

            # per-partition scans along the slot (free) axis
            l1s = sb.tile([U, P], f32)
            nc.vector.tensor_tensor_scan(
                l1s[:], key1[:], key1[:], 0.0, Alu.max, Alu.max
            )
            l2s = sb.tile([U, P], f32)
            nc.vector.tensor_tensor_scan(
                l2s[:], key2[:], key2[:], 0.0, Alu.max, Alu.max
            )
            r1s = sb.tile([U, P], f32)
            nc.vector.tensor_tensor_scan(
                r1s[:, P - 1 :: -1],
                r1[:, P - 1 :: -1],
                r1[:, P - 1 :: -1],
                R_SENT,
                Alu.min,
                Alu.min,
            )
            r2s = sb.tile([U, P], f32)
            nc.vector.tensor_tensor_scan(
                r2s[:, P - 1 :: -1],
                r2[:, P - 1 :: -1],
                r2[:, P - 1 :: -1],
                R_SENT,
                Alu.min,
                Alu.min,
            )

            # cross-partition carry via transposed exclusive scans
            totl = sb.tile([U, 2], f32)
            nc.vector.tensor_copy(totl[:, 0:1], l1s[:, P - 1 : P])
            nc.vector.tensor_copy(totl[:, 1:2], l2s[:, P - 1 : P])
            totr = sb.tile([U, 2], f32)
            nc.vector.tensor_copy(totr[:, 0:1], r1s[:, 0:1])
            nc.vector.tensor_copy(totr[:, 1:2], r2s[:, 0:1])
            tl_ps2 = ps.tile([2, U], f32)
            nc.tensor.transpose(tl_ps2[:], totl[:], idf_t[0:U, 0:U])
            tr_ps = ps.tile([2, U], f32)
            nc.tensor.transpose(tr_ps[:], totr[:], idf_t[0:U, 0:U])
            tlt = sb.tile([2, U], f32)
            nc.vector.tensor_copy(tlt[:], tl_ps2[:])
            trt = sb.tile([2, U], f32)
            nc.vector.tensor_copy(trt[:], tr_ps[:])
            exl = sb.tile([2, U], f32)
            nc.vector.memset(exl[:, 0:1], 0.0)
            nc.vector.tensor_tensor_scan(
                exl[:, 1:U],
                tlt[:, 0 : U - 1],
                tlt[:, 0 : U - 1],
                0.0,
                Alu.max,
                Alu.max,
            )
            exr = sb.tile([2, U], f32)
            nc.vector.memset(exr[:, U - 1 : U], R_SENT)
            nc.vector.tensor_tensor_scan(
                exr[:, U - 2 :: -1],
                trt[:, U - 1 : 0 : -1],
                trt[:, U - 1 : 0 : -1],
                R_SENT,
                Alu.min,
                Alu.min,
            )
            cl_ps = ps.tile([U, 2], f32)
            nc.tensor.transpose(cl_ps[:], exl[:], idf_t[0:2, 0:2])
            cr_ps = ps.tile([U, 2], f32)
            nc.tensor.transpose(cr_ps[:], exr[:], idf_t[0:2, 0:2])
            cl = sb.tile([U, 2], f32)
            nc.vector.tensor_copy(cl[:], cl_ps[:])
            cr = sb.tile([U, 2], f32)
            nc.vector.tensor_copy(cr[:], cr_ps[:])
            nc.vector.tensor_tensor(
                l1s[:], l1s[:], cl[:, 0:1].to_broadcast([U, P]), Alu.max
            )
            nc.vector.tensor_tensor(
                l2s[:], l2s[:], cl[:, 1:2].to_broadcast([U, P]), Alu.max
            )
            nc.vector.tensor_tensor(
                r1s[:], r1s[:], cr[:, 0:1].to_broadcast([U, P]), Alu.min
            )
            nc.vector.tensor_tensor(
                r2s[:], r2s[:], cr[:, 1:2].to_broadcast([U, P]), Alu.min
            )

            # decode: slot = key1>>7, j = ((key1&127)<<6) | (key2&63)
            l1i = sb.tile([U, P], i32)
            nc.vector.tensor_copy(l1i[:], l1s[:])
            l2i = sb.tile([U, P], i32)
            nc.vector.tensor_copy(l2i[:], l2s[:])
            r1i = sb.tile([U, P], i32)
            nc.vector.tensor_copy(r1i[:], r1s[:])
            r2i = sb.tile([U, P], i32)
            nc.vector.tensor_copy(r2i[:], r2s[:])

            slot_l = sb.tile([U, P], i32)
            nc.vector.tensor_scalar(slot_l[:], l1i[:], 7, None, Alu.arith_shift_right)
            slot_r = sb.tile([U, P], i32)
            nc.vector.tensor_scalar(slot_r[:], r1i[:], 7, None, Alu.arith_shift_right)
            jhl = sb.tile([U, P], i32)
            nc.vector.tensor_scalar(
                jhl[:], l1i[:], 127, 6, Alu.bitwise_and, Alu.arith_shift_left
            )
            jll = sb.tile([U, P], i32)
            nc.vector.tensor_scalar(jll[:], l2i[:], 63, None, Alu.bitwise_and)
            jl = sb.tile([U, P], i32)
            nc.vector.tensor_tensor(jl[:], jhl[:], jll[:], Alu.bitwise_or)
            jhr = sb.tile([U, P], i32)
            nc.vector.tensor_scalar(
                jhr[:], r1i[:], 127, 6, Alu.bitwise_and, Alu.arith_shift_left
            )
            jlr = sb.tile([U, P], i32)
            nc.vector.tensor_scalar(jlr[:], r2i[:], 63, None, Alu.bitwise_and)
            jr = sb.tile([U, P], i32)
            nc.vector.tensor_tensor(jr[:], jhr[:], jlr[:], Alu.bitwise_or)

            dl = sb.tile([U, P], i32)
            nc.vector.tensor_tensor(dl[:], pos_t[:], slot_l[:], Alu.subtract)
            drr = sb.tile([U, P], i32)
            nc.vector.tensor_tensor(drr[:], slot_r[:], pos_t[:], Alu.subtract)
            m_l = sb.tile([U, P], i32)
            nc.vector.tensor_tensor(m_l[:], dl[:], drr[:], Alu.is_lt)
            m_r = sb.tile([U, P], i32)
            nc.vector.tensor_tensor(m_r[:], drr[:], dl[:], Alu.is_lt)
            src = sb.tile([U, P], i32)
            nc.vector.tensor_tensor(src[:], jl[:], jr[:], Alu.min)
            nc.vector.copy_predicated(src[:], m_r[:], jr[:])
            nc.vector.copy_predicated(src[:], m_l[:], jl[:])

            # own slots live on partitions 1..16; shift to 0..15 via a PE
            # transpose pair (free-dim slice of the transposed table)
            srcf = sb.tile([U, P], f32)
            nc.vector.tensor_copy(srcf[:], src[:])
            g_ps = ps.tile([P, U], f32)
            nc.tensor.transpose(g_ps[:], srcf[:], idf_t[0:U, 0:U])
            g_sb = sb.tile([P, U], f32)
            nc.vector.tensor_copy(g_sb[:], g_ps[:])
            # dma_gather's 8 DSP cores each read their own 16-partition group:
            # replicate the 16 own-slot columns 8x, then one transpose lands
            # idxs[16c+p, s] = src[p+1, s] for every DSP core c
            g_rep = sb.tile([P, 8, 16], f32)
            nc.vector.tensor_copy(
                g_rep[:], g_sb[:, 1:17].unsqueeze(1).to_broadcast([P, 8, 16])
            )
            ix_ps = ps.tile([P, P], f32)
            nc.tensor.transpose(
                ix_ps[:], g_rep[:].rearrange("a b c -> a (b c)"), idf_t[:]
            )
            idxs = sb.tile([P, P], i16)
            nc.vector.tensor_copy(idxs[:], ix_ps[:])

            # gather + write out, 4 pipelined quarters
            NQ = 4
            SQ = P // NQ  # 32 idx columns -> 512 rows per quarter
            for q in range(NQ):
                gt = sb.tile([P, 4, C], f32, tag=f"g{q}")
                nc.gpsimd.dma_gather(
                    out_ap=gt[:],
                    in_ap=xc[:],
                    idxs_ap=idxs[:, q * SQ : (q + 1) * SQ],
                    num_idxs=4 * P,
                    num_idxs_reg=4 * P,
                    elem_size=C,
                    queue_num=q % 2,
                )
                for cc in range(4):
                    cg = 4 * q + cc
                    weng = nc.sync if cc % 2 == 0 else nc.scalar
                    weng.dma_start(
                        y[:, cg : cg + 1, :, :].squeeze(1).transpose([1, 0, 2]),
                        gt[:, cc, :],
                    )

    return {"y": y}


def host_inputs(x_coarse, keep_idx):
    import ml_dtypes

    bf = ml_dtypes.bfloat16
    x_coarse = np.ascontiguousarray(np.asarray(x_coarse), dtype=np.float32)
    keep = np.ascontiguousarray(np.asarray(keep_idx)).astype(np.int64).reshape(-1)

    iotau = np.tile(np.arange(U, dtype=np.float64), (P, 1))

    in_maps = []
    for m in range(N_CORES):
        base = SLICE * m - P
        hi = (keep >> 7) - (16 * m - 1)
        jj = np.nonzero((hi >= 0) & (hi < U))[0]
        f = (keep[jj] & 127).astype(np.int64)
        order = np.argsort(f, kind="stable")
        fo = f[order]
        jo = jj[order]
        cnt = np.bincount(fo, minlength=P)
        assert cnt.max() <= B, f"bucket overflow: {cnt.max()} > {B}"
        start = np.zeros(P, dtype=np.int64)
        start[1:] = np.cumsum(cnt)[:-1]
        slot_in_bucket = np.arange(len(jo)) - start[fo]
        hi_b = np.full((P, B), 255.0)
        v1 = np.zeros((P, B))
        v2 = np.zeros((P, B))
        hi_b[fo, slot_in_bucket] = hi[jo]
        v1[fo, slot_in_bucket] = (jo >> 6) + 1
        v2[fo, slot_in_bucket] = jo & 63
        pk = np.concatenate([hi_b, v1, v2, iotau], axis=1).astype(bf)
        pos = (
            16384
            + base
            + 128 * np.arange(U, dtype=np.int64)[:, None]
            + np.arange(P, dtype=np.int64)[None, :]
        ).astype(np.int32)
        in_maps.append(
            {
                "xc": x_coarse,
                "pk": np.ascontiguousarray(pk),
                "posw": np.ascontiguousarray(pos),
            }
        )
    return in_maps


def _get_nc():
    if "nc" in _NC_CACHE:
        return _NC_CACHE["nc"]
    _ensure_paths()
    from concourse import bass, mybir
    import concourse.bacc as bacc
    import concourse.tile as tile

    nc = bacc.Bacc(
        "TRN2",
        target_bir_lowering=False,
        debug=False,
        dynamic_dma_scratch_size=16384,
        num_swdge_queues=2,
    )
    build_program(nc, bass, mybir, tile)
    nc.compile()
    _NC_CACHE["nc"] = nc
    return nc


def run_on_hw(in_maps, trace=False, **kwargs):
    _ensure_paths()
    from concourse.bass_utils import run_bass_kernel_spmd

    nc = _get_nc()
    return run_bass_kernel_spmd(
        nc, in_maps, core_ids=list(range(N_CORES)), trace=trace, **kwargs
    )


def kernel(x_coarse, keep_idx, E_fine=None, **_unused):
    in_maps = host_inputs(x_coarse, keep_idx)
    res = run_on_hw(in_maps)
    out = np.concatenate(
        [res.results[m]["y"].reshape(SLICE, C) for m in range(N_CORES)], axis=0
    )
    return np.ascontiguousarray(out.astype(np.float32, copy=False))
